# revision 10
# baseline (speedup 1.0000x reference)
"""Causal self-attention (B=4, T=2048, C=1024, H=16) on 8 TRN2 NeuronCores.

Sharding: tensor-parallel over heads. Each core owns 2 heads:
  Launch A (per core): QKV^T projection for its 2 heads (fp32r matmuls),
    flash-style causal attention with softmax computed in the S^T layout
    (k on partitions, q on free dim; rowsums via a ones-column in V'),
    normalized per-head output y_heads^T [B, 128, T].
  Host: concatenate the 8 per-core head outputs (pure gather) and re-shard
    by token slices.
  Launch B (per core): c_proj for a 1024-token slice (full C contraction)
    + bias -> final [tokens, C] slice. Host concatenates slices.

All matmuls use fp32r (tf32-like, ~1.5e-4 rel err per 128-contraction,
1 cycle/row on the PE for moving dim >= 256). No host FLOPs: the host only
transposes/slices/concatenates.
"""

import os
import time
from contextlib import ExitStack

import numpy as np

import concourse.bass as bass
import concourse.tile as tile
from concourse import bacc, mybir
from concourse.bass_utils import run_bass_kernel_spmd
from concourse.masks import make_identity

B, T, C = 4, 2048, 1024
H, D = 16, 64
NCORES = 8
HPC = H // NCORES            # heads per core = 2
HD = HPC * D                 # per-core head feature width = 128
F32 = mybir.dt.float32
F32R = mybir.dt.float32r

QT = 512                     # q tile (moving free dim)
KT = 128                     # k tile (S^T partition dim)
NQT = T // QT                # 4
NKT = T // KT                # 16

_CACHE = {}

TRACE = os.environ.get("KERNEL_TRACE", "0") == "1"
LAST_EXEC_NS = {}


def _build_launch_a():
    nc = bacc.Bacc("TRN2", target_bir_lowering=False, debug=False)

    xt_d = nc.dram_tensor("xt", [B, C, T], F32R, kind="ExternalInput").ap()
    w_d = nc.dram_tensor("wqkv", [C, 3 * HD], F32R, kind="ExternalInput").ap()
    b_d = nc.dram_tensor("bqkv", [3 * HD], F32, kind="ExternalInput").ap()
    yt_d = nc.dram_tensor("yt", [B, HD, T], F32, kind="ExternalOutput").ap()

    with tile.TileContext(nc) as tc, ExitStack() as ctx:
        consts = ctx.enter_context(tc.tile_pool(name="consts", bufs=1))
        xt_pool = ctx.enter_context(tc.tile_pool(name="xt", bufs=9))
        qkvt_pool = ctx.enter_context(tc.tile_pool(name="qkvt", bufs=2))
        vn_pool = ctx.enter_context(tc.tile_pool(name="vn", bufs=2))
        es_pool = ctx.enter_context(tc.tile_pool(name="es", bufs=4))
        y_pool = ctx.enter_context(tc.tile_pool(name="y", bufs=2))
        small = ctx.enter_context(tc.tile_pool(name="small", bufs=2))
        psA = ctx.enter_context(tc.tile_pool(name="psA", bufs=4, space="PSUM"))
        psS = ctx.enter_context(tc.tile_pool(name="psS", bufs=2, space="PSUM"))

        # --- constants ---
        w_sb = consts.tile([128, 8, 3 * HD], F32R)     # [p, ct, f]
        nc.sync.dma_start(w_sb[:], w_d.rearrange("(ct p) f -> p ct f", p=128))
        b_sb = consts.tile([128, 3], F32)              # per-partition bias per ftile
        nc.sync.dma_start(b_sb[:], b_d.rearrange("(ft p) -> p ft", p=128))
        ident_f = consts.tile([128, 128], F32)
        make_identity(nc, ident_f[:])
        ident = consts.tile([128, 128], F32R)
        nc.vector.tensor_copy(ident[:], ident_f[:])
        ones64_f = consts.tile([1, 64], F32)
        nc.vector.memset(ones64_f[:], 1.0)
        ones64 = consts.tile([1, 64], F32R)
        nc.vector.tensor_copy(ones64[:], ones64_f[:])
        onescol_f = consts.tile([128, NKT], F32)
        nc.vector.memset(onescol_f[:], 1.0)

        for b in range(B):
            # --- load X^T for this batch ---
            xts = []
            for ct in range(8):
                t = xt_pool.tile([128, T], F32R, tag="xt")
                nc.sync.dma_start(t[:], xt_d[b, ct * 128:(ct + 1) * 128, :])
                xts.append(t)

            # --- QKV^T = Wc^T @ X (per ftile: q, k, v) ---
            qkvt = qkvt_pool.tile([128, 3, T], F32R)
            for ft in range(3):
                pss = [psA.tile([128, QT], F32, tag="psA", name=f"qkvps{_i}") for _i in range(4)]
                for ct in range(8):
                    lhsT = w_sb[:, ct, ft * 128:(ft + 1) * 128]
                    for tt in range(4):
                        nc.tensor.matmul(
                            pss[tt][:], lhsT, xts[ct][:, tt * QT:(tt + 1) * QT],
                            start=(ct == 0), stop=(ct == 7),
                        )
                for tt in range(4):
                    nc.vector.tensor_scalar_add(
                        qkvt[:, ft, tt * QT:(tt + 1) * QT], pss[tt][:],
                        b_sb[:, ft:ft + 1],
                    )

            # --- V natural [k, (V_h0|1|V_h1|1)] via PE transpose ---
            vn = vn_pool.tile([128, NKT, 130], F32R)
            nc.vector.tensor_copy(vn[:, :, 64], onescol_f[:])
            nc.vector.tensor_copy(vn[:, :, 129], onescol_f[:])
            for kt in range(NKT):
                trp = psA.tile([128, 128], F32R, tag="psA")
                nc.tensor.transpose(trp[:], qkvt[:, 2, kt * 128:(kt + 1) * 128], ident[:])
                nc.vector.tensor_copy(vn[:, kt, 0:64], trp[:, 0:64])
                nc.vector.tensor_copy(vn[:, kt, 65:129], trp[:, 64:128])

            # --- attention per head / q-tile ---
            y_sb = y_pool.tile([HD, T], F32)
            for h in range(HPC):
                hp = slice(h * 64, (h + 1) * 64)
                for qi in range(NQT):
                    nkt = 4 * (qi + 1)            # causal: only k-tiles <= q range
                    qsl = slice(qi * QT, (qi + 1) * QT)
                    o_ps = psA.tile([65, QT], F32, tag="psA")
                    for ktp in range((nkt + 1) // 2):
                        kts = [k for k in (2 * ktp, 2 * ktp + 1) if k < nkt]
                        s_ps = psS.tile([128, 1024], F32, tag="psS")
                        for j, kt in enumerate(kts):
                            nc.tensor.matmul(
                                s_ps[:, j * QT:(j + 1) * QT],
                                qkvt[hp, 1, kt * 128:(kt + 1) * 128],
                                qkvt[hp, 0, qsl],
                                start=True, stop=True,
                            )
                        es = es_pool.tile([128, 1024], F32R, tag="es")
                        nc.scalar.activation(
                            out=es[:, 0:len(kts) * QT], in_=s_ps[:, 0:len(kts) * QT],
                            func=mybir.ActivationFunctionType.Exp, scale=0.125,
                        )
                        for j, kt in enumerate(kts):
                            if kt >= nkt - 4:
                                # diagonal tile: zero where k > q
                                # keep (kt*128 + p) <= (qi*512 + qf)
                                nc.gpsimd.affine_select(
                                    out=es[:, j * QT:(j + 1) * QT],
                                    in_=es[:, j * QT:(j + 1) * QT],
                                    compare_op=mybir.AluOpType.is_ge,
                                    fill=0.0,
                                    base=qi * QT - kt * 128,
                                    pattern=[[1, QT]],
                                    channel_multiplier=-1,
                                )
                        for j, kt in enumerate(kts):
                            nc.tensor.matmul(
                                o_ps[:],
                                vn[:, kt, h * 65:(h + 1) * 65],
                                es[:, j * QT:(j + 1) * QT],
                                start=(kt == 0), stop=(kt == nkt - 1),
                            )
                    # normalize: y = O_unnorm / rowsum (broadcast via K=1 matmul)
                    rcp = small.tile([1, QT], F32R, tag="rcp")
                    with nc.allow_low_precision(reason="f32r is bit-identical to f32"):
                        nc.vector.reciprocal(rcp[:], o_ps[64:65, :])
                    bc_ps = psA.tile([64, QT], F32, tag="psA")
                    nc.tensor.matmul(bc_ps[:], ones64[:], rcp[:], start=True, stop=True)
                    bc_sb = small.tile([64, QT], F32, tag="bc")
                    nc.scalar.copy(bc_sb[:], bc_ps[:])
                    nc.vector.tensor_mul(y_sb[hp, qsl], o_ps[0:64, :], bc_sb[:])
            nc.sync.dma_start(yt_d[b], y_sb[:])

    nc.compile()
    return nc


def _build_launch_b():
    nc = bacc.Bacc("TRN2", target_bir_lowering=False, debug=False)

    TB = B * T // NCORES     # 1024 tokens per core
    yt_d = nc.dram_tensor("ytc", [C, TB], F32R, kind="ExternalInput").ap()
    w_d = nc.dram_tensor("wp", [C, C], F32R, kind="ExternalInput").ap()
    b_d = nc.dram_tensor("bp", [C], F32, kind="ExternalInput").ap()
    o_d = nc.dram_tensor("out", [TB, C], F32, kind="ExternalOutput").ap()

    with tile.TileContext(nc) as tc, ExitStack() as ctx:
        consts = ctx.enter_context(tc.tile_pool(name="consts", bufs=1))
        outp = ctx.enter_context(tc.tile_pool(name="outp", bufs=3))
        ps = ctx.enter_context(tc.tile_pool(name="ps", bufs=4, space="PSUM"))

        w_sb = consts.tile([128, 8, C], F32R)
        nc.sync.dma_start(w_sb[:], w_d.rearrange("(ct p) f -> p ct f", p=128))
        y_sb = consts.tile([128, 8, TB], F32R)
        nc.sync.dma_start(y_sb[:], yt_d.rearrange("(ct p) t -> p ct t", p=128))
        bias = consts.tile([128, C], F32)
        nc.gpsimd.dma_start(
            out=bias[:], in_=bass.AP(tensor=b_d.tensor, offset=0, ap=[[0, 128], [1, C]])
        )

        for m in range(TB // 128):
            pss = [ps.tile([128, 512], F32, tag="ps", name=f"prps{_i}") for _i in range(2)]
            for ct in range(8):
                lhsT = y_sb[:, ct, m * 128:(m + 1) * 128]
                for n in range(2):
                    nc.tensor.matmul(
                        pss[n][:], lhsT, w_sb[:, ct, n * 512:(n + 1) * 512],
                        start=(ct == 0), stop=(ct == 7),
                    )
            o_sb = outp.tile([128, C], F32, tag="o")
            for n in range(2):
                nc.vector.tensor_add(
                    o_sb[:, n * 512:(n + 1) * 512], pss[n][:],
                    bias[:, n * 512:(n + 1) * 512],
                )
            nc.sync.dma_start(o_d[m * 128:(m + 1) * 128, :], o_sb[:])

    nc.compile()
    return nc


def kernel(x, W_attn, b_attn, W_proj, b_proj):
    x = np.asarray(x, dtype=np.float32)
    W_attn = np.asarray(W_attn, dtype=np.float32)
    b_attn = np.asarray(b_attn, dtype=np.float32)
    W_proj = np.asarray(W_proj, dtype=np.float32)
    b_proj = np.asarray(b_proj, dtype=np.float32)

    if "a" not in _CACHE:
        _CACHE["a"] = _build_launch_a()
    if "b" not in _CACHE:
        _CACHE["b"] = _build_launch_b()

    # ---- host prep: transpose/slice only (no FLOPs) ----
    xt = np.ascontiguousarray(x.transpose(0, 2, 1))          # [B, C, T]

    in_a = []
    for c in range(NCORES):
        sl = slice(c * HD, (c + 1) * HD)
        w = np.ascontiguousarray(
            np.concatenate(
                [W_attn[:, sl], W_attn[:, C:][:, sl], W_attn[:, 2 * C:][:, sl]],
                axis=1,
            )
        )
        bq = np.concatenate([b_attn[sl], b_attn[C:][sl], b_attn[2 * C:][sl]])
        in_a.append({"xt": xt, "wqkv": w, "bqkv": np.ascontiguousarray(bq)})

    t0 = time.time()
    ra = run_bass_kernel_spmd(_CACHE["a"], in_a, core_ids=list(range(NCORES)))
    LAST_EXEC_NS["a_wall"] = int((time.time() - t0) * 1e9)
    yts = [r["yt"] for r in ra.results]                      # each [B, HD, T]
    ytf = np.concatenate(yts, axis=1)                        # [B, C, T]

    in_b = []
    for c in range(NCORES):
        bidx, thalf = c // 2, c % 2
        ytc = np.ascontiguousarray(ytf[bidx, :, thalf * 1024:(thalf + 1) * 1024])
        in_b.append({"ytc": ytc, "wp": W_proj, "bp": b_proj})

    t0 = time.time()
    rb = run_bass_kernel_spmd(_CACHE["b"], in_b, core_ids=list(range(NCORES)))
    LAST_EXEC_NS["b_wall"] = int((time.time() - t0) * 1e9)

    out = np.empty((B, T, C), dtype=np.float32)
    for c in range(NCORES):
        bidx, thalf = c // 2, c % 2
        out[bidx, thalf * 1024:(thalf + 1) * 1024, :] = rb.results[c]["out"]
    return out
